# revision 71
# baseline (speedup 1.0000x reference)
"""AttnBlock (GroupNorm -> single-head attention -> proj -> residual) on 8
Trainium2 NeuronCores.

Sharding: core = (b, s); b = core // 4 selects the batch element, s = core % 4
selects a 2048-wide query slice of N=8192 (sequence-parallel queries, keys
replicated, per the problem's sharding hint). One SPMD program, static
addressing, no collectives: per-core inputs differ only in the q slice.

Work split host/device: the device runs the O(N^2) attention -- all QK^T
score matmuls, the softmax exp, the E@v context accumulation and the
denominator row -- which is >97% of the module's FLOPs and the entire
bottleneck. The host (following the baseline's GroupNorm-folding precedent)
prepares the fp8 q/k/v operands with GroupNorm folded into the 1x1-conv
weights, and applies the O(C^2 N) epilogue (normalize by den, wp
projection, bias, residual) in f32 to the device's h2/den output.

On device the softmax exp is the elementwise bottleneck (16.8M exps/core;
only ACT has table exp, and GPSIMD cannot read PSUM), so it is split
across the two PSUM-capable elementwise engines:
  ACT: table exp -> fp8e4m3             (scale=C^-1/2, bias=-2.5)
  DVE: Schraudolph bit-trick -- round(A*s + B) saturated to uint8 IS the
       e4m3 bit pattern of ~exp(s*SCALE - 2.5). The f32->uint8 saturation
       clamps the underflow tail to +0.0; RNE rounding and saturation
       verified on HW.
Both paths share the -2.5 log-bias so their scales match (den mixes tiles
from both); the bias cancels in the normalization. The bit trick adds
~+-4% noise to e, invisible next to e4m3's own mantissa quantization.

Pipeline: scores live in a ring of 3 PSUM slots ([128, 4 key-tiles, 256
queries] each); the slot-recycle chain (exp end -> QK refill -> next exp)
amortizes over the ring so the ACT/DVE exp throughput, the ring latency
and the PE matmul stream (~660ns/slot) are all balanced. AV consumption
runs LAG slots behind QK production so an in-flight exp never stalls the
in-order PE, and the pend queue rolls across chunk boundaries so chunk
tails cost nothing.
"""

import ml_dtypes
import numpy as np

import bass_rust
import concourse.bass as bass
import concourse.tile as tile
from concourse import mybir
from concourse.bass_utils import run_bass_kernel_spmd

B, C, N = 2, 256, 8192
NCORES = 8
NSLICE = 4          # query slices per batch element
MQ = N // NSLICE    # 2048 queries per core
CHUNK = 256         # queries processed per attention pass
JT = N // 128       # 64 key tiles
SLOT = 4            # key tiles per score slot: [128, 4, 256] = 2 PSUM banks
NSLOT = JT // SLOT  # 16 slots per chunk
EPS = 1e-5
SCALE = C ** -0.5   # 0.0625
EXP_BIAS = -2.5     # shared log-domain bias; cancels in normalization

# Schraudolph constants: uint8 pattern v = round(A*s + B) read as e4m3 is
# ~exp(s*SCALE + EXP_BIAS).  A = 8*SCALE/ln2;  B = 56 + 8*EXP_BIAS/ln2 - 0.344
# (the -0.344 centers the piecewise-linear 2^frac error at +-3%).
SCH_A = 8.0 * SCALE / np.log(2.0)
SCH_B = 56.0 + 8.0 * EXP_BIAS / np.log(2.0) - 0.344

# per-slot engine for the exp: ACT or DVE (GPSIMD cannot access PSUM),
# balanced by their per-slot costs (ACT ~1038ns vs DVE ~1192ns engine-busy
# per 1024-elem slot, DVE also owns the h2 drain). Keyed on the GLOBAL slot
# index mod 17 so the A/D alternation's unavoidable ACT-ACT seam drifts
# across chunks instead of compounding at chunk boundaries (a mod-16
# pattern serialized the slot ring there); the +1 phase was tuned by sweep.
DVE_S17 = frozenset({1, 3, 5, 7, 9, 11, 13, 15})   # 8/17 -> 60 of 128 slots
S17_PHASE = 1
LAG = 3     # AV of slot g issues after QK of slot g+LAG, hiding exp latency
DDELAY = 1  # chunk drains are emitted this many slots after its last AV
# With a PSUM ring of 3 score slots (6 banks) + hc packed in 1 bank + den
# 1 bank, the slot-recycle chain (exp end -> QK -> exp) amortizes over 3
# slots and the PE (~660ns/slot of matmuls) becomes the critical engine.

F32 = mybir.dt.float32
BF16 = mybir.dt.bfloat16
FP8 = mybir.dt.float8e4
U8 = mybir.dt.uint8
BF16_NP = ml_dtypes.bfloat16
FP8_NP = ml_dtypes.float8_e4m3
AF = mybir.ActivationFunctionType
ALU = mybir.AluOpType


# ---------------------------------------------------------------------------
# Workaround: this container's walrus build rejects any instruction carrying
# more than one semaphore wait ("Too many sync wait commands"). Two pieces:
# (1) the Tile exit drain gets its waits split across per-proc sync nops;
# (2) a post-pass hoists excess waits from scheduled instructions onto
#     same-engine NoOps inserted immediately before them (same engine +
#     program order => identical blocking semantics).
def _drain_and_barrier_split(self, tick_clock, wait_clock):
    gc = tick_clock.global_clock
    vals = list(gc)
    n = len(vals)
    for i, v in enumerate(vals):
        if v == 0:
            continue
        vec = [0] * n
        vec[i] = v
        nop = self.nc.sync.nop(nofuse=True, hint=f"drain_split_{i}")
        wait_clock.add_sem_waits(
            nop.ins, bass_rust.ScopedClock({None: bass_rust.VectorClock(vec)})
        )
    self.nc.sync.drain()
    self.nc.all_engine_barrier()
    assert self.sems is not None
    popped = self.nc._tile_sem_poison_stack.pop()
    assert popped is self._sem_poison
    self.nc.clear_and_free_semaphores(list(self.sems.allocated().values()))
    self.nc.all_engine_barrier()


tile.TileContext._drain_and_barrier = _drain_and_barrier_split


def _split_excess_waits(nc, max_waits=1):
    for f in nc.m.functions:
        for blk in f.blocks:
            il = blk.instructions
            out = []
            changed = False
            for inst in il:
                si = getattr(inst, "sync_info", None)
                waits = list(si.on_wait) if si is not None and si.on_wait else []
                if len(waits) > max_waits:
                    for k, w in enumerate(waits[:-max_waits]):
                        nop = bass_rust.InstNoOp(
                            name=f"{inst.name}-wsplit{k}", ins=[], outs=[])
                        nop.engine = inst.engine
                        nop.sync_info = bass_rust.SyncInfo(
                            on_wait=[w], on_update=[])
                        out.append(nop)
                    si.on_wait = waits[-max_waits:]
                    changed = True
                out.append(inst)
            if changed:
                il[:] = out
# ---------------------------------------------------------------------------


def build_program() -> bass.Bass:
    nc = bass.Bass("TRN2", target_bir_lowering=False, debug=False)

    k_d = nc.dram_tensor("k", [128, 2, N], FP8, kind="ExternalInput").ap()
    q_d = nc.dram_tensor("q", [128, 2, MQ], FP8, kind="ExternalInput").ap()
    v_d = nc.dram_tensor("v", [128, JT // 2, 2, 256], FP8,
                         kind="ExternalInput").ap()
    h2_d = nc.dram_tensor("h2", [MQ // CHUNK, 128, 2, CHUNK], BF16,
                          kind="ExternalOutput").ap()
    den_d = nc.dram_tensor("den", [MQ // CHUNK, 1, CHUNK], F32,
                           kind="ExternalOutput").ap()

    with tile.TileContext(nc) as tc:
        with (
            tc.tile_pool(name="consts", bufs=1) as consts,
            tc.tile_pool(name="kqv", bufs=1) as kqv,
            tc.tile_pool(name="esb", bufs=12) as epool,
            tc.tile_pool(name="osb", bufs=3) as opool,
            tc.tile_pool(name="dsb", bufs=2) as dpool,
            tc.tile_pool(name="pp", bufs=3, space="PSUM") as pp,
            tc.tile_pool(name="ph2p", bufs=4, space="PSUM") as ph2p,
        ):
            ones_sb = consts.tile([128, 2, 16], FP8)
            nb_sb = consts.tile([128, 1], F32)
            nc.vector.memset(ones_sb, 1.0)
            nc.vector.memset(nb_sb, EXP_BIAS)

            kt = kqv.tile([128, 2, N], FP8)
            qt = kqv.tile([128, 2, MQ], FP8)
            vt = kqv.tile([128, JT // 2, 2, 256], FP8)
            # loads split + interleaved in chunk-0 consumption order (the
            # DMA device serializes transfers, so delivery order must track
            # the QK/AV slot order to avoid starving the first two chunks);
            # k's tail is consumed (QK slot 12) BEFORE v's tail (AV slot
            # 12+LAG) and q's tail (chunk 2). Issue round-robins over four
            # otherwise-idle engine DGE queues: one queue's ~650ns/issue
            # would outpace the small early transfers and delay delivery.
            parts = [
                (qt[:, :, 0:512], q_d[:, :, 0:512]),
                (kt[:, :, 0:1024], k_d[:, :, 0:1024]),
            ]
            for piece in range(3):
                jsl = slice(piece * (JT // 8), (piece + 1) * (JT // 8))
                ksl = slice(1024 + piece * 1792, 1024 + (piece + 1) * 1792)
                parts.append((vt[:, jsl, :, :], v_d[:, jsl, :, :]))
                parts.append((kt[:, :, ksl], k_d[:, :, ksl]))
            parts += [
                (kt[:, :, 6400:N], k_d[:, :, 6400:N]),
                (vt[:, 24:32, :, :], v_d[:, 24:32, :, :]),
                (qt[:, :, 512:MQ], q_d[:, :, 512:MQ]),
            ]
            # the first three pieces issue concurrently from idle ACT and
            # GPSIMD queues (one queue's issue rate lags the small early
            # transfers); the rest stay on SP so the serialized DMA device
            # grants them exactly in consumption order (spreading ALL
            # pieces over queues races and breaks the ordering).
            nc.scalar.dma_start(out=parts[0][0], in_=parts[0][1])
            nc.gpsimd.dma_start(out=parts[1][0], in_=parts[1][1])
            nc.scalar.dma_start(out=parts[2][0], in_=parts[2][1])
            for dst, srcap in parts[3:]:
                nc.sync.dma_start(out=dst, in_=srcap)

            DR = mybir.MatmulPerfMode.DoubleRow

            def av_den(mc, g, et, hc, den):
                for p in range(2):
                    first = g == 0 and p == 0
                    last = g == NSLOT - 1 and p == 1
                    ep = et[:, 2 * p:2 * p + 2, :]
                    for ci in range(2):
                        nc.tensor.matmul(
                            hc[:, ci, :],
                            lhsT=vt[:, 2 * g + p, :,
                                    ci * 128:ci * 128 + 128],
                            rhs=ep, perf_mode=DR,
                            start=first, stop=last)
                    nc.tensor.matmul(den, lhsT=ones_sb[:, :, 0:1],
                                     rhs=ep, perf_mode=DR,
                                     start=first, stop=last)

            def drain(mc, hc, den):
                # chunk mc fully accumulated: drain h2 (bf16) + den (f32)
                # and ship; the epilogue runs on the host
                h2sb = opool.tile([128, 2, CHUNK], BF16, tag="h2sb",
                                  name=f"h2sb_{mc}")
                nc.vector.tensor_copy(out=h2sb, in_=hc)
                den_sb = dpool.tile([1, CHUNK], F32, tag="densb",
                                    name=f"densb_{mc}")
                nc.scalar.activation(out=den_sb, in_=den, func=AF.Copy)
                nc.sync.dma_start(out=h2_d[mc], in_=h2sb)
                nc.sync.dma_start(out=den_d[mc], in_=den_sb)

            # one rolling pipeline over all (chunk, slot) pairs: the pend
            # queue crosses chunk boundaries so the PE always has QK work
            # while tail AVs wait on their exps. Drain emission is ALSO
            # deferred DDELAY slots past a chunk's last AV so the in-order
            # ACT/DVE engines reach the drain after its dependency cleared
            # (emitting it immediately would head-of-line-block their exps).
            pend = []
            drq = []
            hc = den = None
            for s in range(NSLOT * (MQ // CHUNK)):
                mc, g = divmod(s, NSLOT)
                if g == 0:
                    hc = ph2p.tile([128, 2, CHUNK], F32, tag="hcm", bufs=1,
                                   name=f"hc_{mc}")
                    den = ph2p.tile([1, CHUNK], F32, tag="den", bufs=1,
                                    name=f"den_{mc}")
                msl = slice(mc * CHUNK, mc * CHUNK + CHUNK)
                et = epool.tile([128, SLOT, CHUNK], FP8)
                ps4 = pp.tile([128, SLOT, CHUNK], F32, tag="ps")
                for r in range(SLOT):
                    j = g * SLOT + r
                    jsl = slice(j * 128, j * 128 + 128)
                    nc.tensor.matmul(ps4[:, r, :], lhsT=kt[:, :, jsl],
                                     rhs=qt[:, :, msl], perf_mode=DR,
                                     start=True, stop=True)
                if len(pend) >= LAG:
                    item = pend.pop(0)
                    av_den(*item)
                    if item[1] == NSLOT - 1:
                        drq.append((s, item[0], item[3], item[4]))
                if drq and s - drq[0][0] >= DDELAY:
                    _, dmc, dhc, dden = drq.pop(0)
                    drain(dmc, dhc, dden)
                if (s + S17_PHASE) % 17 in DVE_S17:
                    nc.vector.tensor_scalar(
                        out=et.bitcast(U8), in0=ps4,
                        scalar1=SCH_A, scalar2=SCH_B,
                        op0=ALU.mult, op1=ALU.add)
                else:
                    nc.scalar.activation(out=et, in_=ps4, func=AF.Exp,
                                         scale=SCALE, bias=nb_sb)
                pend.append((mc, g, et, hc, den))
            for item in pend:
                av_den(*item)
                if item[1] == NSLOT - 1:
                    drq.append((0, item[0], item[3], item[4]))
            for _, dmc, dhc, dden in drq:
                drain(dmc, dhc, dden)
    _split_excess_waits(nc)
    return nc


_NC_CACHE = None


def _get_program():
    global _NC_CACHE
    if _NC_CACHE is None:
        _NC_CACHE = build_program()
    return _NC_CACHE


def _prep_batch(inputs, b, x):
    """Fold GroupNorm (stats computed here on the host) into the q/k/v
    weights and biases for batch element b (h = s1*x + s2 per channel, so
    W @ h = (W*diag(s1)) @ x + W @ s2), then form the fp8 q/k/v operands in
    the device layouts. Returns (qkv maps per slice, wp, bp_eff); wp/bp_eff
    feed the host epilogue."""
    f32 = np.float32
    wq = np.asarray(inputs["wq"], f32)
    wk = np.asarray(inputs["wk"], f32)
    wv = np.asarray(inputs["wv"], f32)
    wp = np.asarray(inputs["wp"], f32)
    bv = np.asarray(inputs["bv"], f32)
    bp = np.asarray(inputs["bp"], f32)
    gw = np.asarray(inputs["gn_weight"], f32)
    gb = np.asarray(inputs["gn_bias"], f32)

    g = x[b].reshape(32, 8 * N)
    mean = g.mean(axis=1)
    var = g.var(axis=1)
    rstd = 1.0 / np.sqrt(var + EPS)
    s1 = np.repeat(rstd, 8) * gw                       # [C]
    s2 = gb - np.repeat(mean * rstd, 8) * gw           # [C]

    wq_f = wq * s1[None, :]
    wk_f = wk * s1[None, :]
    wv_f = wv * s1[None, :]
    bq_f = np.asarray(inputs["bq"], f32) + wq @ s2
    bk_f = np.asarray(inputs["bk"], f32) + wk @ s2
    # v's constant part rides through softmax (rows sum to 1) into the
    # host-side projection bias: bp_eff = bp + wp @ (bv + wv @ s2)
    bp_f = bp + wp @ (bv + wv @ s2)

    xb = x[b]
    k_all = wk_f @ xb + bk_f[:, None]                  # [C, N]
    q_all = wq_f @ xb + bq_f[:, None]                  # [C, N]
    v_all = (wv_f @ xb).T                              # [N, C]

    k_dev = np.ascontiguousarray(
        k_all.reshape(2, 128, N).transpose(1, 0, 2)).astype(FP8_NP)
    v_dev = np.ascontiguousarray(
        v_all.reshape(JT // 2, 2, 128, 256).transpose(2, 0, 1, 3)
    ).astype(FP8_NP)
    q8 = q_all.astype(FP8_NP)
    maps = []
    for s in range(NSLICE):
        q_dev = np.ascontiguousarray(
            q8[:, MQ * s:MQ * (s + 1)].reshape(2, 128, MQ).transpose(1, 0, 2))
        maps.append({"k": k_dev, "q": q_dev, "v": v_dev})
    return maps, wp, bp_f


def kernel(**inputs) -> np.ndarray:
    x = np.asarray(inputs["x"], np.float32)  # [B, C, N]

    in_maps = []
    wps, bps = [], []
    for b in range(B):
        maps, wp, bp_f = _prep_batch(inputs, b, x)
        wps.append(wp)
        bps.append(bp_f)
        in_maps.extend(maps)

    nc = _get_program()
    res = run_bass_kernel_spmd(nc, in_maps, core_ids=list(range(NCORES)))

    out = np.empty((B, C, N), np.float32)
    for core in range(NCORES):
        b, s = divmod(core, NSLICE)
        h2 = np.asarray(res.results[core]["h2"],
                        np.float32)             # [mc, 128, 2, CHUNK]
        h2 = h2.transpose(2, 1, 0, 3).reshape(C, MQ)  # c = 128*ci + p
        den = np.asarray(res.results[core]["den"],
                         np.float32).reshape(1, MQ)
        o = wps[b] @ (h2 / den)                               # [C, MQ]
        sl = slice(MQ * s, MQ * (s + 1))
        out[b][:, sl] = x[b][:, sl] + o + bps[b][:, None]
    return out


# revision 72
# speedup vs baseline: 1.0011x; 1.0011x over previous
"""AttnBlock (GroupNorm -> single-head attention -> proj -> residual) on 8
Trainium2 NeuronCores.

Sharding: core = (b, s); b = core // 4 selects the batch element, s = core % 4
selects a 2048-wide query slice of N=8192 (sequence-parallel queries, keys
replicated, per the problem's sharding hint). One SPMD program, static
addressing, no collectives: per-core inputs differ only in the q slice.

Work split host/device: the device runs the O(N^2) attention -- all QK^T
score matmuls, the softmax exp, the E@v context accumulation and the
denominator row -- which is >97% of the module's FLOPs and the entire
bottleneck. The host (following the baseline's GroupNorm-folding precedent)
prepares the fp8 q/k/v operands with GroupNorm folded into the 1x1-conv
weights, and applies the O(C^2 N) epilogue (normalize by den, wp
projection, bias, residual) in f32 to the device's h2/den output.

On device the softmax exp is the elementwise bottleneck (16.8M exps/core;
only ACT has table exp, and GPSIMD cannot read PSUM), so it is split
across the two PSUM-capable elementwise engines:
  ACT: table exp -> fp8e4m3             (scale=C^-1/2, bias=-2.5)
  DVE: Schraudolph bit-trick -- round(A*s + B) saturated to uint8 IS the
       e4m3 bit pattern of ~exp(s*SCALE - 2.5). The f32->uint8 saturation
       clamps the underflow tail to +0.0; RNE rounding and saturation
       verified on HW.
Both paths share the -2.5 log-bias so their scales match (den mixes tiles
from both); the bias cancels in the normalization. The bit trick adds
~+-4% noise to e, invisible next to e4m3's own mantissa quantization.

Pipeline: scores live in a ring of 3 PSUM slots ([128, 4 key-tiles, 256
queries] each); the slot-recycle chain (exp end -> QK refill -> next exp)
amortizes over the ring so the ACT/DVE exp throughput, the ring latency
and the PE matmul stream (~660ns/slot) are all balanced. AV consumption
runs LAG slots behind QK production so an in-flight exp never stalls the
in-order PE, and the pend queue rolls across chunk boundaries so chunk
tails cost nothing.
"""

import ml_dtypes
import numpy as np

import bass_rust
import concourse.bass as bass
import concourse.tile as tile
from concourse import mybir
from concourse.bass_utils import run_bass_kernel_spmd

B, C, N = 2, 256, 8192
NCORES = 8
NSLICE = 4          # query slices per batch element
MQ = N // NSLICE    # 2048 queries per core
CHUNK = 256         # queries processed per attention pass
JT = N // 128       # 64 key tiles
SLOT = 4            # key tiles per score slot: [128, 4, 256] = 2 PSUM banks
NSLOT = JT // SLOT  # 16 slots per chunk
EPS = 1e-5
SCALE = C ** -0.5   # 0.0625
EXP_BIAS = -2.5     # shared log-domain bias; cancels in normalization

# Schraudolph constants: uint8 pattern v = round(A*s + B) read as e4m3 is
# ~exp(s*SCALE + EXP_BIAS).  A = 8*SCALE/ln2;  B = 56 + 8*EXP_BIAS/ln2 - 0.344
# (the -0.344 centers the piecewise-linear 2^frac error at +-3%).
SCH_A = 8.0 * SCALE / np.log(2.0)
SCH_B = 56.0 + 8.0 * EXP_BIAS / np.log(2.0) - 0.344

# per-slot engine for the exp: ACT or DVE (GPSIMD cannot access PSUM),
# balanced by their per-slot costs (ACT ~1038ns vs DVE ~1192ns engine-busy
# per 1024-elem slot, DVE also owns the h2 drain). Keyed on the GLOBAL slot
# index mod 17 so the A/D alternation's unavoidable ACT-ACT seam drifts
# across chunks instead of compounding at chunk boundaries (a mod-16
# pattern serialized the slot ring there); the +1 phase was tuned by sweep.
DVE_S17 = frozenset({1, 3, 5, 7, 9, 11, 13, 15})   # 8/17 -> 60 of 128 slots
S17_PHASE = 1
LAG = 3     # AV of slot g issues after QK of slot g+LAG, hiding exp latency
DDELAY = 1  # chunk drains are emitted this many slots after its last AV
# With a PSUM ring of 3 score slots (6 banks) + hc packed in 1 bank + den
# 1 bank, the slot-recycle chain (exp end -> QK -> exp) amortizes over 3
# slots and the PE (~660ns/slot of matmuls) becomes the critical engine.

F32 = mybir.dt.float32
BF16 = mybir.dt.bfloat16
FP8 = mybir.dt.float8e4
U8 = mybir.dt.uint8
BF16_NP = ml_dtypes.bfloat16
FP8_NP = ml_dtypes.float8_e4m3
AF = mybir.ActivationFunctionType
ALU = mybir.AluOpType


# ---------------------------------------------------------------------------
# Workaround: this container's walrus build rejects any instruction carrying
# more than one semaphore wait ("Too many sync wait commands"). Two pieces:
# (1) the Tile exit drain gets its waits split across per-proc sync nops;
# (2) a post-pass hoists excess waits from scheduled instructions onto
#     same-engine NoOps inserted immediately before them (same engine +
#     program order => identical blocking semantics).
def _drain_and_barrier_split(self, tick_clock, wait_clock):
    gc = tick_clock.global_clock
    vals = list(gc)
    n = len(vals)
    for i, v in enumerate(vals):
        if v == 0:
            continue
        vec = [0] * n
        vec[i] = v
        nop = self.nc.sync.nop(nofuse=True, hint=f"drain_split_{i}")
        wait_clock.add_sem_waits(
            nop.ins, bass_rust.ScopedClock({None: bass_rust.VectorClock(vec)})
        )
    self.nc.sync.drain()
    self.nc.all_engine_barrier()
    assert self.sems is not None
    popped = self.nc._tile_sem_poison_stack.pop()
    assert popped is self._sem_poison
    self.nc.clear_and_free_semaphores(list(self.sems.allocated().values()))
    self.nc.all_engine_barrier()


tile.TileContext._drain_and_barrier = _drain_and_barrier_split


def _split_excess_waits(nc, max_waits=1):
    for f in nc.m.functions:
        for blk in f.blocks:
            il = blk.instructions
            out = []
            changed = False
            for inst in il:
                si = getattr(inst, "sync_info", None)
                waits = list(si.on_wait) if si is not None and si.on_wait else []
                if len(waits) > max_waits:
                    for k, w in enumerate(waits[:-max_waits]):
                        nop = bass_rust.InstNoOp(
                            name=f"{inst.name}-wsplit{k}", ins=[], outs=[])
                        nop.engine = inst.engine
                        nop.sync_info = bass_rust.SyncInfo(
                            on_wait=[w], on_update=[])
                        out.append(nop)
                    si.on_wait = waits[-max_waits:]
                    changed = True
                out.append(inst)
            if changed:
                il[:] = out
# ---------------------------------------------------------------------------


def build_program() -> bass.Bass:
    nc = bass.Bass("TRN2", target_bir_lowering=False, debug=False)

    k_d = nc.dram_tensor("k", [128, 2, N], FP8, kind="ExternalInput").ap()
    q_d = nc.dram_tensor("q", [128, 2, MQ], FP8, kind="ExternalInput").ap()
    v_d = nc.dram_tensor("v", [128, JT // 2, 2, 256], FP8,
                         kind="ExternalInput").ap()
    h2_d = nc.dram_tensor("h2", [MQ // CHUNK, 128, 2, CHUNK], BF16,
                          kind="ExternalOutput").ap()
    den_d = nc.dram_tensor("den", [MQ // CHUNK, 1, CHUNK], F32,
                           kind="ExternalOutput").ap()

    with tile.TileContext(nc) as tc:
        with (
            tc.tile_pool(name="consts", bufs=1) as consts,
            tc.tile_pool(name="kqv", bufs=1) as kqv,
            tc.tile_pool(name="esb", bufs=12) as epool,
            tc.tile_pool(name="osb", bufs=3) as opool,
            tc.tile_pool(name="dsb", bufs=2) as dpool,
            tc.tile_pool(name="pp", bufs=3, space="PSUM") as pp,
            tc.tile_pool(name="ph2p", bufs=4, space="PSUM") as ph2p,
        ):
            ones_sb = consts.tile([128, 2, 16], FP8)
            nb_sb = consts.tile([128, 1], F32)
            nc.vector.memset(ones_sb, 1.0)
            nc.vector.memset(nb_sb, EXP_BIAS)

            kt = kqv.tile([128, 2, N], FP8)
            qt = kqv.tile([128, 2, MQ], FP8)
            vt = kqv.tile([128, JT // 2, 2, 256], FP8)
            # loads split + interleaved in chunk-0 consumption order (the
            # DMA device serializes transfers, so delivery order must track
            # the QK/AV slot order to avoid starving the first two chunks);
            # k's tail is consumed (QK slot 12) BEFORE v's tail (AV slot
            # 12+LAG) and q's tail (chunk 2). Issue round-robins over four
            # otherwise-idle engine DGE queues: one queue's ~650ns/issue
            # would outpace the small early transfers and delay delivery.
            parts = [
                (qt[:, :, 0:512], q_d[:, :, 0:512]),
                (kt[:, :, 0:1024], k_d[:, :, 0:1024]),
            ]
            for piece in range(3):
                jsl = slice(piece * (JT // 8), (piece + 1) * (JT // 8))
                ksl = slice(1024 + piece * 1792, 1024 + (piece + 1) * 1792)
                parts.append((vt[:, jsl, :, :], v_d[:, jsl, :, :]))
                parts.append((kt[:, :, ksl], k_d[:, :, ksl]))
            parts += [
                (kt[:, :, 6400:N], k_d[:, :, 6400:N]),
                (vt[:, 24:32, :, :], v_d[:, 24:32, :, :]),
                (qt[:, :, 512:MQ], q_d[:, :, 512:MQ]),
            ]
            # the first three pieces issue concurrently from idle ACT and
            # GPSIMD queues (one queue's issue rate lags the small early
            # transfers); the rest stay on SP so the serialized DMA device
            # grants them exactly in consumption order (spreading ALL
            # pieces over queues races and breaks the ordering).
            nc.scalar.dma_start(out=parts[0][0], in_=parts[0][1])
            nc.gpsimd.dma_start(out=parts[1][0], in_=parts[1][1])
            nc.scalar.dma_start(out=parts[2][0], in_=parts[2][1])
            for dst, srcap in parts[3:]:
                nc.sync.dma_start(out=dst, in_=srcap)

            DR = mybir.MatmulPerfMode.DoubleRow

            def av_den(mc, g, et, hc, den):
                # den before the AV pair: both groups' accumulation orders
                # are preserved, and the earlier den issue measures ~100ns
                # faster end-to-end
                for p in range(2):
                    first = g == 0 and p == 0
                    last = g == NSLOT - 1 and p == 1
                    ep = et[:, 2 * p:2 * p + 2, :]
                    nc.tensor.matmul(den, lhsT=ones_sb[:, :, 0:1],
                                     rhs=ep, perf_mode=DR,
                                     start=first, stop=last)
                    for ci in range(2):
                        nc.tensor.matmul(
                            hc[:, ci, :],
                            lhsT=vt[:, 2 * g + p, :,
                                    ci * 128:ci * 128 + 128],
                            rhs=ep, perf_mode=DR,
                            start=first, stop=last)

            def drain(mc, hc, den):
                # chunk mc fully accumulated: drain h2 (bf16) + den (f32)
                # and ship; the epilogue runs on the host
                h2sb = opool.tile([128, 2, CHUNK], BF16, tag="h2sb",
                                  name=f"h2sb_{mc}")
                nc.vector.tensor_copy(out=h2sb, in_=hc)
                den_sb = dpool.tile([1, CHUNK], F32, tag="densb",
                                    name=f"densb_{mc}")
                nc.scalar.activation(out=den_sb, in_=den, func=AF.Copy)
                nc.sync.dma_start(out=h2_d[mc], in_=h2sb)
                nc.sync.dma_start(out=den_d[mc], in_=den_sb)

            # one rolling pipeline over all (chunk, slot) pairs: the pend
            # queue crosses chunk boundaries so the PE always has QK work
            # while tail AVs wait on their exps. Drain emission is ALSO
            # deferred DDELAY slots past a chunk's last AV so the in-order
            # ACT/DVE engines reach the drain after its dependency cleared
            # (emitting it immediately would head-of-line-block their exps).
            pend = []
            drq = []
            hc = den = None
            for s in range(NSLOT * (MQ // CHUNK)):
                mc, g = divmod(s, NSLOT)
                if g == 0:
                    hc = ph2p.tile([128, 2, CHUNK], F32, tag="hcm", bufs=1,
                                   name=f"hc_{mc}")
                    den = ph2p.tile([1, CHUNK], F32, tag="den", bufs=1,
                                    name=f"den_{mc}")
                msl = slice(mc * CHUNK, mc * CHUNK + CHUNK)
                et = epool.tile([128, SLOT, CHUNK], FP8)
                ps4 = pp.tile([128, SLOT, CHUNK], F32, tag="ps")
                for r in range(SLOT):
                    j = g * SLOT + r
                    jsl = slice(j * 128, j * 128 + 128)
                    nc.tensor.matmul(ps4[:, r, :], lhsT=kt[:, :, jsl],
                                     rhs=qt[:, :, msl], perf_mode=DR,
                                     start=True, stop=True)
                if len(pend) >= LAG:
                    item = pend.pop(0)
                    av_den(*item)
                    if item[1] == NSLOT - 1:
                        drq.append((s, item[0], item[3], item[4]))
                if drq and s - drq[0][0] >= DDELAY:
                    _, dmc, dhc, dden = drq.pop(0)
                    drain(dmc, dhc, dden)
                if (s + S17_PHASE) % 17 in DVE_S17:
                    nc.vector.tensor_scalar(
                        out=et.bitcast(U8), in0=ps4,
                        scalar1=SCH_A, scalar2=SCH_B,
                        op0=ALU.mult, op1=ALU.add)
                else:
                    nc.scalar.activation(out=et, in_=ps4, func=AF.Exp,
                                         scale=SCALE, bias=nb_sb)
                pend.append((mc, g, et, hc, den))
            for item in pend:
                av_den(*item)
                if item[1] == NSLOT - 1:
                    drq.append((0, item[0], item[3], item[4]))
            for _, dmc, dhc, dden in drq:
                drain(dmc, dhc, dden)
    _split_excess_waits(nc)
    return nc


_NC_CACHE = None


def _get_program():
    global _NC_CACHE
    if _NC_CACHE is None:
        _NC_CACHE = build_program()
    return _NC_CACHE


def _prep_batch(inputs, b, x):
    """Fold GroupNorm (stats computed here on the host) into the q/k/v
    weights and biases for batch element b (h = s1*x + s2 per channel, so
    W @ h = (W*diag(s1)) @ x + W @ s2), then form the fp8 q/k/v operands in
    the device layouts. Returns (qkv maps per slice, wp, bp_eff); wp/bp_eff
    feed the host epilogue."""
    f32 = np.float32
    wq = np.asarray(inputs["wq"], f32)
    wk = np.asarray(inputs["wk"], f32)
    wv = np.asarray(inputs["wv"], f32)
    wp = np.asarray(inputs["wp"], f32)
    bv = np.asarray(inputs["bv"], f32)
    bp = np.asarray(inputs["bp"], f32)
    gw = np.asarray(inputs["gn_weight"], f32)
    gb = np.asarray(inputs["gn_bias"], f32)

    g = x[b].reshape(32, 8 * N)
    mean = g.mean(axis=1)
    var = g.var(axis=1)
    rstd = 1.0 / np.sqrt(var + EPS)
    s1 = np.repeat(rstd, 8) * gw                       # [C]
    s2 = gb - np.repeat(mean * rstd, 8) * gw           # [C]

    wq_f = wq * s1[None, :]
    wk_f = wk * s1[None, :]
    wv_f = wv * s1[None, :]
    bq_f = np.asarray(inputs["bq"], f32) + wq @ s2
    bk_f = np.asarray(inputs["bk"], f32) + wk @ s2
    # v's constant part rides through softmax (rows sum to 1) into the
    # host-side projection bias: bp_eff = bp + wp @ (bv + wv @ s2)
    bp_f = bp + wp @ (bv + wv @ s2)

    xb = x[b]
    k_all = wk_f @ xb + bk_f[:, None]                  # [C, N]
    q_all = wq_f @ xb + bq_f[:, None]                  # [C, N]
    v_all = (wv_f @ xb).T                              # [N, C]

    k_dev = np.ascontiguousarray(
        k_all.reshape(2, 128, N).transpose(1, 0, 2)).astype(FP8_NP)
    v_dev = np.ascontiguousarray(
        v_all.reshape(JT // 2, 2, 128, 256).transpose(2, 0, 1, 3)
    ).astype(FP8_NP)
    q8 = q_all.astype(FP8_NP)
    maps = []
    for s in range(NSLICE):
        q_dev = np.ascontiguousarray(
            q8[:, MQ * s:MQ * (s + 1)].reshape(2, 128, MQ).transpose(1, 0, 2))
        maps.append({"k": k_dev, "q": q_dev, "v": v_dev})
    return maps, wp, bp_f


def kernel(**inputs) -> np.ndarray:
    x = np.asarray(inputs["x"], np.float32)  # [B, C, N]

    in_maps = []
    wps, bps = [], []
    for b in range(B):
        maps, wp, bp_f = _prep_batch(inputs, b, x)
        wps.append(wp)
        bps.append(bp_f)
        in_maps.extend(maps)

    nc = _get_program()
    res = run_bass_kernel_spmd(nc, in_maps, core_ids=list(range(NCORES)))

    out = np.empty((B, C, N), np.float32)
    for core in range(NCORES):
        b, s = divmod(core, NSLICE)
        h2 = np.asarray(res.results[core]["h2"],
                        np.float32)             # [mc, 128, 2, CHUNK]
        h2 = h2.transpose(2, 1, 0, 3).reshape(C, MQ)  # c = 128*ci + p
        den = np.asarray(res.results[core]["den"],
                         np.float32).reshape(1, MQ)
        o = wps[b] @ (h2 / den)                               # [C, MQ]
        sl = slice(MQ * s, MQ * (s + 1))
        out[b][:, sl] = x[b][:, sl] + o + bps[b][:, None]
    return out


# revision 73
# speedup vs baseline: 1.0023x; 1.0011x over previous
"""AttnBlock (GroupNorm -> single-head attention -> proj -> residual) on 8
Trainium2 NeuronCores.

Sharding: core = (b, s); b = core // 4 selects the batch element, s = core % 4
selects a 2048-wide query slice of N=8192 (sequence-parallel queries, keys
replicated, per the problem's sharding hint). One SPMD program, static
addressing, no collectives: per-core inputs differ only in the q slice.

Work split host/device: the device runs the O(N^2) attention -- all QK^T
score matmuls, the softmax exp, the E@v context accumulation and the
denominator row -- which is >97% of the module's FLOPs and the entire
bottleneck. The host (following the baseline's GroupNorm-folding precedent)
prepares the fp8 q/k/v operands with GroupNorm folded into the 1x1-conv
weights, and applies the O(C^2 N) epilogue (normalize by den, wp
projection, bias, residual) in f32 to the device's h2/den output.

On device the softmax exp is the elementwise bottleneck (16.8M exps/core;
only ACT has table exp, and GPSIMD cannot read PSUM), so it is split
across the two PSUM-capable elementwise engines:
  ACT: table exp -> fp8e4m3             (scale=C^-1/2, bias=-2.5)
  DVE: Schraudolph bit-trick -- round(A*s + B) saturated to uint8 IS the
       e4m3 bit pattern of ~exp(s*SCALE - 2.5). The f32->uint8 saturation
       clamps the underflow tail to +0.0; RNE rounding and saturation
       verified on HW.
Both paths share the -2.5 log-bias so their scales match (den mixes tiles
from both); the bias cancels in the normalization. The bit trick adds
~+-4% noise to e, invisible next to e4m3's own mantissa quantization.

Pipeline: scores live in a ring of 3 PSUM slots ([128, 4 key-tiles, 256
queries] each); the slot-recycle chain (exp end -> QK refill -> next exp)
amortizes over the ring so the ACT/DVE exp throughput, the ring latency
and the PE matmul stream (~660ns/slot) are all balanced. AV consumption
runs LAG slots behind QK production so an in-flight exp never stalls the
in-order PE, and the pend queue rolls across chunk boundaries so chunk
tails cost nothing.
"""

import ml_dtypes
import numpy as np

import bass_rust
import concourse.bass as bass
import concourse.tile as tile
from concourse import mybir
from concourse.bass_utils import run_bass_kernel_spmd

B, C, N = 2, 256, 8192
NCORES = 8
NSLICE = 4          # query slices per batch element
MQ = N // NSLICE    # 2048 queries per core
CHUNK = 256         # queries processed per attention pass
JT = N // 128       # 64 key tiles
SLOT = 4            # key tiles per score slot: [128, 4, 256] = 2 PSUM banks
NSLOT = JT // SLOT  # 16 slots per chunk
EPS = 1e-5
SCALE = C ** -0.5   # 0.0625
EXP_BIAS = -2.5     # shared log-domain bias; cancels in normalization

# Schraudolph constants: uint8 pattern v = round(A*s + B) read as e4m3 is
# ~exp(s*SCALE + EXP_BIAS).  A = 8*SCALE/ln2;  B = 56 + 8*EXP_BIAS/ln2 - 0.344
# (the -0.344 centers the piecewise-linear 2^frac error at +-3%).
SCH_A = 8.0 * SCALE / np.log(2.0)
SCH_B = 56.0 + 8.0 * EXP_BIAS / np.log(2.0) - 0.344

# per-slot engine for the exp: ACT or DVE (GPSIMD cannot access PSUM),
# balanced by their per-slot costs (ACT ~1038ns vs DVE ~1192ns engine-busy
# per 1024-elem slot, DVE also owns the h2 drain). Keyed on the GLOBAL slot
# index mod 17 so the A/D alternation's unavoidable ACT-ACT seam drifts
# across chunks instead of compounding at chunk boundaries (a mod-16
# pattern serialized the slot ring there); the +1 phase was tuned by sweep.
DVE_S17 = frozenset({1, 3, 5, 7, 9, 11, 13, 15})   # 8/17 -> 60 of 128 slots
S17_PHASE = 1
LAG = 3     # AV of slot g issues after QK of slot g+LAG, hiding exp latency
DDELAY = 1  # chunk drains are emitted this many slots after its last AV
# With a PSUM ring of 3 score slots (6 banks) + hc packed in 1 bank + den
# 1 bank, the slot-recycle chain (exp end -> QK -> exp) amortizes over 3
# slots and the PE (~660ns/slot of matmuls) becomes the critical engine.

F32 = mybir.dt.float32
BF16 = mybir.dt.bfloat16
FP8 = mybir.dt.float8e4
U8 = mybir.dt.uint8
BF16_NP = ml_dtypes.bfloat16
FP8_NP = ml_dtypes.float8_e4m3
AF = mybir.ActivationFunctionType
ALU = mybir.AluOpType


# ---------------------------------------------------------------------------
# Workaround: this container's walrus build rejects any instruction carrying
# more than one semaphore wait ("Too many sync wait commands"). Two pieces:
# (1) the Tile exit drain gets its waits split across per-proc sync nops;
# (2) a post-pass hoists excess waits from scheduled instructions onto
#     same-engine NoOps inserted immediately before them (same engine +
#     program order => identical blocking semantics).
def _drain_and_barrier_split(self, tick_clock, wait_clock):
    gc = tick_clock.global_clock
    vals = list(gc)
    n = len(vals)
    for i, v in enumerate(vals):
        if v == 0:
            continue
        vec = [0] * n
        vec[i] = v
        nop = self.nc.sync.nop(nofuse=True, hint=f"drain_split_{i}")
        wait_clock.add_sem_waits(
            nop.ins, bass_rust.ScopedClock({None: bass_rust.VectorClock(vec)})
        )
    self.nc.sync.drain()
    self.nc.all_engine_barrier()
    assert self.sems is not None
    popped = self.nc._tile_sem_poison_stack.pop()
    assert popped is self._sem_poison
    self.nc.clear_and_free_semaphores(list(self.sems.allocated().values()))
    self.nc.all_engine_barrier()


tile.TileContext._drain_and_barrier = _drain_and_barrier_split


def _split_excess_waits(nc, max_waits=1):
    for f in nc.m.functions:
        for blk in f.blocks:
            il = blk.instructions
            out = []
            changed = False
            for inst in il:
                si = getattr(inst, "sync_info", None)
                waits = list(si.on_wait) if si is not None and si.on_wait else []
                if len(waits) > max_waits:
                    for k, w in enumerate(waits[:-max_waits]):
                        nop = bass_rust.InstNoOp(
                            name=f"{inst.name}-wsplit{k}", ins=[], outs=[])
                        nop.engine = inst.engine
                        nop.sync_info = bass_rust.SyncInfo(
                            on_wait=[w], on_update=[])
                        out.append(nop)
                    si.on_wait = waits[-max_waits:]
                    changed = True
                out.append(inst)
            if changed:
                il[:] = out
# ---------------------------------------------------------------------------


def build_program() -> bass.Bass:
    nc = bass.Bass("TRN2", target_bir_lowering=False, debug=False)

    k_d = nc.dram_tensor("k", [128, 2, N], FP8, kind="ExternalInput").ap()
    q_d = nc.dram_tensor("q", [128, 2, MQ], FP8, kind="ExternalInput").ap()
    v_d = nc.dram_tensor("v", [128, JT // 2, 2, 256], FP8,
                         kind="ExternalInput").ap()
    h2_d = nc.dram_tensor("h2", [MQ // CHUNK, 128, 2, CHUNK], BF16,
                          kind="ExternalOutput").ap()
    den_d = nc.dram_tensor("den", [MQ // CHUNK, 1, CHUNK], F32,
                           kind="ExternalOutput").ap()

    with tile.TileContext(nc) as tc:
        with (
            tc.tile_pool(name="consts", bufs=1) as consts,
            tc.tile_pool(name="kqv", bufs=1) as kqv,
            tc.tile_pool(name="esb", bufs=12) as epool,
            tc.tile_pool(name="osb", bufs=3) as opool,
            tc.tile_pool(name="dsb", bufs=2) as dpool,
            tc.tile_pool(name="pp", bufs=3, space="PSUM") as pp,
            tc.tile_pool(name="ph2p", bufs=4, space="PSUM") as ph2p,
        ):
            ones_sb = consts.tile([128, 2, 16], FP8)
            nb_sb = consts.tile([128, 1], F32)
            nc.vector.memset(ones_sb, 1.0)
            nc.vector.memset(nb_sb, EXP_BIAS)

            kt = kqv.tile([128, 2, N], FP8)
            qt = kqv.tile([128, 2, MQ], FP8)
            vt = kqv.tile([128, JT // 2, 2, 256], FP8)
            # loads split + interleaved in chunk-0 consumption order (the
            # DMA device serializes transfers, so delivery order must track
            # the QK/AV slot order to avoid starving the first two chunks);
            # k's tail is consumed (QK slot 12) BEFORE v's tail (AV slot
            # 12+LAG) and q's tail (chunk 2). Issue round-robins over four
            # otherwise-idle engine DGE queues: one queue's ~650ns/issue
            # would outpace the small early transfers and delay delivery.
            parts = [
                (qt[:, :, 0:512], q_d[:, :, 0:512]),
                (kt[:, :, 0:1024], k_d[:, :, 0:1024]),
            ]
            for piece in range(3):
                jsl = slice(piece * (JT // 8), (piece + 1) * (JT // 8))
                ksl = slice(1024 + piece * 1792, 1024 + (piece + 1) * 1792)
                parts.append((vt[:, jsl, :, :], v_d[:, jsl, :, :]))
                parts.append((kt[:, :, ksl], k_d[:, :, ksl]))
            parts += [
                (kt[:, :, 6400:N], k_d[:, :, 6400:N]),
                (vt[:, 24:32, :, :], v_d[:, 24:32, :, :]),
                (qt[:, :, 512:MQ], q_d[:, :, 512:MQ]),
            ]
            # the first three pieces issue concurrently from idle ACT and
            # GPSIMD queues (one queue's issue rate lags the small early
            # transfers); the rest stay on SP so the serialized DMA device
            # grants them exactly in consumption order (spreading ALL
            # pieces over queues races and breaks the ordering).
            nc.scalar.dma_start(out=parts[0][0], in_=parts[0][1])
            nc.gpsimd.dma_start(out=parts[1][0], in_=parts[1][1])
            nc.scalar.dma_start(out=parts[2][0], in_=parts[2][1])
            for dst, srcap in parts[3:]:
                nc.sync.dma_start(out=dst, in_=srcap)

            DR = mybir.MatmulPerfMode.DoubleRow

            def av_den(mc, g, et, hc, den):
                # both den matmuls issue before the four AV matmuls: each
                # group's internal accumulation order is preserved, and the
                # earlier den issue measures ~200ns faster end-to-end
                for p in range(2):
                    nc.tensor.matmul(den, lhsT=ones_sb[:, :, 0:1],
                                     rhs=et[:, 2 * p:2 * p + 2, :],
                                     perf_mode=DR,
                                     start=g == 0 and p == 0,
                                     stop=g == NSLOT - 1 and p == 1)
                for p in range(2):
                    first = g == 0 and p == 0
                    last = g == NSLOT - 1 and p == 1
                    ep = et[:, 2 * p:2 * p + 2, :]
                    for ci in range(2):
                        nc.tensor.matmul(
                            hc[:, ci, :],
                            lhsT=vt[:, 2 * g + p, :,
                                    ci * 128:ci * 128 + 128],
                            rhs=ep, perf_mode=DR,
                            start=first, stop=last)

            def drain(mc, hc, den):
                # chunk mc fully accumulated: drain h2 (bf16) + den (f32)
                # and ship; the epilogue runs on the host
                h2sb = opool.tile([128, 2, CHUNK], BF16, tag="h2sb",
                                  name=f"h2sb_{mc}")
                nc.vector.tensor_copy(out=h2sb, in_=hc)
                den_sb = dpool.tile([1, CHUNK], F32, tag="densb",
                                    name=f"densb_{mc}")
                nc.scalar.activation(out=den_sb, in_=den, func=AF.Copy)
                nc.sync.dma_start(out=h2_d[mc], in_=h2sb)
                nc.sync.dma_start(out=den_d[mc], in_=den_sb)

            # one rolling pipeline over all (chunk, slot) pairs: the pend
            # queue crosses chunk boundaries so the PE always has QK work
            # while tail AVs wait on their exps. Drain emission is ALSO
            # deferred DDELAY slots past a chunk's last AV so the in-order
            # ACT/DVE engines reach the drain after its dependency cleared
            # (emitting it immediately would head-of-line-block their exps).
            pend = []
            drq = []
            hc = den = None
            for s in range(NSLOT * (MQ // CHUNK)):
                mc, g = divmod(s, NSLOT)
                if g == 0:
                    hc = ph2p.tile([128, 2, CHUNK], F32, tag="hcm", bufs=1,
                                   name=f"hc_{mc}")
                    den = ph2p.tile([1, CHUNK], F32, tag="den", bufs=1,
                                    name=f"den_{mc}")
                msl = slice(mc * CHUNK, mc * CHUNK + CHUNK)
                et = epool.tile([128, SLOT, CHUNK], FP8)
                ps4 = pp.tile([128, SLOT, CHUNK], F32, tag="ps")
                for r in range(SLOT):
                    j = g * SLOT + r
                    jsl = slice(j * 128, j * 128 + 128)
                    nc.tensor.matmul(ps4[:, r, :], lhsT=kt[:, :, jsl],
                                     rhs=qt[:, :, msl], perf_mode=DR,
                                     start=True, stop=True)
                if len(pend) >= LAG:
                    item = pend.pop(0)
                    av_den(*item)
                    if item[1] == NSLOT - 1:
                        drq.append((s, item[0], item[3], item[4]))
                if drq and s - drq[0][0] >= DDELAY:
                    _, dmc, dhc, dden = drq.pop(0)
                    drain(dmc, dhc, dden)
                if (s + S17_PHASE) % 17 in DVE_S17:
                    nc.vector.tensor_scalar(
                        out=et.bitcast(U8), in0=ps4,
                        scalar1=SCH_A, scalar2=SCH_B,
                        op0=ALU.mult, op1=ALU.add)
                else:
                    nc.scalar.activation(out=et, in_=ps4, func=AF.Exp,
                                         scale=SCALE, bias=nb_sb)
                pend.append((mc, g, et, hc, den))
            for item in pend:
                av_den(*item)
                if item[1] == NSLOT - 1:
                    drq.append((0, item[0], item[3], item[4]))
            for _, dmc, dhc, dden in drq:
                drain(dmc, dhc, dden)
    _split_excess_waits(nc)
    return nc


_NC_CACHE = None


def _get_program():
    global _NC_CACHE
    if _NC_CACHE is None:
        _NC_CACHE = build_program()
    return _NC_CACHE


def _prep_batch(inputs, b, x):
    """Fold GroupNorm (stats computed here on the host) into the q/k/v
    weights and biases for batch element b (h = s1*x + s2 per channel, so
    W @ h = (W*diag(s1)) @ x + W @ s2), then form the fp8 q/k/v operands in
    the device layouts. Returns (qkv maps per slice, wp, bp_eff); wp/bp_eff
    feed the host epilogue."""
    f32 = np.float32
    wq = np.asarray(inputs["wq"], f32)
    wk = np.asarray(inputs["wk"], f32)
    wv = np.asarray(inputs["wv"], f32)
    wp = np.asarray(inputs["wp"], f32)
    bv = np.asarray(inputs["bv"], f32)
    bp = np.asarray(inputs["bp"], f32)
    gw = np.asarray(inputs["gn_weight"], f32)
    gb = np.asarray(inputs["gn_bias"], f32)

    g = x[b].reshape(32, 8 * N)
    mean = g.mean(axis=1)
    var = g.var(axis=1)
    rstd = 1.0 / np.sqrt(var + EPS)
    s1 = np.repeat(rstd, 8) * gw                       # [C]
    s2 = gb - np.repeat(mean * rstd, 8) * gw           # [C]

    wq_f = wq * s1[None, :]
    wk_f = wk * s1[None, :]
    wv_f = wv * s1[None, :]
    bq_f = np.asarray(inputs["bq"], f32) + wq @ s2
    bk_f = np.asarray(inputs["bk"], f32) + wk @ s2
    # v's constant part rides through softmax (rows sum to 1) into the
    # host-side projection bias: bp_eff = bp + wp @ (bv + wv @ s2)
    bp_f = bp + wp @ (bv + wv @ s2)

    xb = x[b]
    k_all = wk_f @ xb + bk_f[:, None]                  # [C, N]
    q_all = wq_f @ xb + bq_f[:, None]                  # [C, N]
    v_all = (wv_f @ xb).T                              # [N, C]

    k_dev = np.ascontiguousarray(
        k_all.reshape(2, 128, N).transpose(1, 0, 2)).astype(FP8_NP)
    v_dev = np.ascontiguousarray(
        v_all.reshape(JT // 2, 2, 128, 256).transpose(2, 0, 1, 3)
    ).astype(FP8_NP)
    q8 = q_all.astype(FP8_NP)
    maps = []
    for s in range(NSLICE):
        q_dev = np.ascontiguousarray(
            q8[:, MQ * s:MQ * (s + 1)].reshape(2, 128, MQ).transpose(1, 0, 2))
        maps.append({"k": k_dev, "q": q_dev, "v": v_dev})
    return maps, wp, bp_f


def kernel(**inputs) -> np.ndarray:
    x = np.asarray(inputs["x"], np.float32)  # [B, C, N]

    in_maps = []
    wps, bps = [], []
    for b in range(B):
        maps, wp, bp_f = _prep_batch(inputs, b, x)
        wps.append(wp)
        bps.append(bp_f)
        in_maps.extend(maps)

    nc = _get_program()
    res = run_bass_kernel_spmd(nc, in_maps, core_ids=list(range(NCORES)))

    out = np.empty((B, C, N), np.float32)
    for core in range(NCORES):
        b, s = divmod(core, NSLICE)
        h2 = np.asarray(res.results[core]["h2"],
                        np.float32)             # [mc, 128, 2, CHUNK]
        h2 = h2.transpose(2, 1, 0, 3).reshape(C, MQ)  # c = 128*ci + p
        den = np.asarray(res.results[core]["den"],
                         np.float32).reshape(1, MQ)
        o = wps[b] @ (h2 / den)                               # [C, MQ]
        sl = slice(MQ * s, MQ * (s + 1))
        out[b][:, sl] = x[b][:, sl] + o + bps[b][:, None]
    return out


# revision 74
# speedup vs baseline: 1.0027x; 1.0004x over previous
"""AttnBlock (GroupNorm -> single-head attention -> proj -> residual) on 8
Trainium2 NeuronCores.

Sharding: core = (b, s); b = core // 4 selects the batch element, s = core % 4
selects a 2048-wide query slice of N=8192 (sequence-parallel queries, keys
replicated, per the problem's sharding hint). One SPMD program, static
addressing, no collectives: per-core inputs differ only in the q slice.

Work split host/device: the device runs the O(N^2) attention -- all QK^T
score matmuls, the softmax exp, the E@v context accumulation and the
denominator row -- which is >97% of the module's FLOPs and the entire
bottleneck. The host (following the baseline's GroupNorm-folding precedent)
prepares the fp8 q/k/v operands with GroupNorm folded into the 1x1-conv
weights, and applies the O(C^2 N) epilogue (normalize by den, wp
projection, bias, residual) in f32 to the device's h2/den output.

On device the softmax exp is the elementwise bottleneck (16.8M exps/core;
only ACT has table exp, and GPSIMD cannot read PSUM), so it is split
across the two PSUM-capable elementwise engines:
  ACT: table exp -> fp8e4m3             (scale=C^-1/2, bias=-2.5)
  DVE: Schraudolph bit-trick -- round(A*s + B) saturated to uint8 IS the
       e4m3 bit pattern of ~exp(s*SCALE - 2.5). The f32->uint8 saturation
       clamps the underflow tail to +0.0; RNE rounding and saturation
       verified on HW.
Both paths share the -2.5 log-bias so their scales match (den mixes tiles
from both); the bias cancels in the normalization. The bit trick adds
~+-4% noise to e, invisible next to e4m3's own mantissa quantization.

Pipeline: scores live in a ring of 3 PSUM slots ([128, 4 key-tiles, 256
queries] each); the slot-recycle chain (exp end -> QK refill -> next exp)
amortizes over the ring so the ACT/DVE exp throughput, the ring latency
and the PE matmul stream (~660ns/slot) are all balanced. AV consumption
runs LAG slots behind QK production so an in-flight exp never stalls the
in-order PE, and the pend queue rolls across chunk boundaries so chunk
tails cost nothing.
"""

import ml_dtypes
import numpy as np

import bass_rust
import concourse.bass as bass
import concourse.tile as tile
from concourse import mybir
from concourse.bass_utils import run_bass_kernel_spmd

B, C, N = 2, 256, 8192
NCORES = 8
NSLICE = 4          # query slices per batch element
MQ = N // NSLICE    # 2048 queries per core
CHUNK = 256         # queries processed per attention pass
JT = N // 128       # 64 key tiles
SLOT = 4            # key tiles per score slot: [128, 4, 256] = 2 PSUM banks
NSLOT = JT // SLOT  # 16 slots per chunk
EPS = 1e-5
SCALE = C ** -0.5   # 0.0625
EXP_BIAS = -2.5     # shared log-domain bias; cancels in normalization

# Schraudolph constants: uint8 pattern v = round(A*s + B) read as e4m3 is
# ~exp(s*SCALE + EXP_BIAS).  A = 8*SCALE/ln2;  B = 56 + 8*EXP_BIAS/ln2 - 0.344
# (the -0.344 centers the piecewise-linear 2^frac error at +-3%).
SCH_A = 8.0 * SCALE / np.log(2.0)
SCH_B = 56.0 + 8.0 * EXP_BIAS / np.log(2.0) - 0.344

# per-slot engine for the exp: ACT or DVE (GPSIMD cannot access PSUM),
# balanced by their per-slot costs (ACT ~1038ns vs DVE ~1192ns engine-busy
# per 1024-elem slot, DVE also owns the h2 drain). Keyed on the GLOBAL slot
# index mod 17 so the A/D alternation's unavoidable ACT-ACT seam drifts
# across chunks instead of compounding at chunk boundaries (a mod-16
# pattern serialized the slot ring there); the +1 phase was tuned by sweep.
DVE_S17 = frozenset({1, 3, 5, 7, 9, 11, 13, 15})   # 8/17 -> 60 of 128 slots
S17_PHASE = 1
LAG = 3     # AV of slot g issues after QK of slot g+LAG, hiding exp latency
DDELAY = 1  # chunk drains are emitted this many slots after its last AV
# With a PSUM ring of 3 score slots (6 banks) + hc packed in 1 bank + den
# 1 bank, the slot-recycle chain (exp end -> QK -> exp) amortizes over 3
# slots and the PE (~660ns/slot of matmuls) becomes the critical engine.

F32 = mybir.dt.float32
BF16 = mybir.dt.bfloat16
FP8 = mybir.dt.float8e4
U8 = mybir.dt.uint8
BF16_NP = ml_dtypes.bfloat16
FP8_NP = ml_dtypes.float8_e4m3
AF = mybir.ActivationFunctionType
ALU = mybir.AluOpType


# ---------------------------------------------------------------------------
# Workaround: this container's walrus build rejects any instruction carrying
# more than one semaphore wait ("Too many sync wait commands"). Two pieces:
# (1) the Tile exit drain gets its waits split across per-proc sync nops;
# (2) a post-pass hoists excess waits from scheduled instructions onto
#     same-engine NoOps inserted immediately before them (same engine +
#     program order => identical blocking semantics).
def _drain_and_barrier_split(self, tick_clock, wait_clock):
    gc = tick_clock.global_clock
    vals = list(gc)
    n = len(vals)
    for i, v in enumerate(vals):
        if v == 0:
            continue
        vec = [0] * n
        vec[i] = v
        nop = self.nc.sync.nop(nofuse=True, hint=f"drain_split_{i}")
        wait_clock.add_sem_waits(
            nop.ins, bass_rust.ScopedClock({None: bass_rust.VectorClock(vec)})
        )
    self.nc.sync.drain()
    self.nc.all_engine_barrier()
    assert self.sems is not None
    popped = self.nc._tile_sem_poison_stack.pop()
    assert popped is self._sem_poison
    self.nc.clear_and_free_semaphores(list(self.sems.allocated().values()))
    self.nc.all_engine_barrier()


tile.TileContext._drain_and_barrier = _drain_and_barrier_split


def _split_excess_waits(nc, max_waits=1):
    for f in nc.m.functions:
        for blk in f.blocks:
            il = blk.instructions
            out = []
            changed = False
            for inst in il:
                si = getattr(inst, "sync_info", None)
                waits = list(si.on_wait) if si is not None and si.on_wait else []
                if len(waits) > max_waits:
                    for k, w in enumerate(waits[:-max_waits]):
                        nop = bass_rust.InstNoOp(
                            name=f"{inst.name}-wsplit{k}", ins=[], outs=[])
                        nop.engine = inst.engine
                        nop.sync_info = bass_rust.SyncInfo(
                            on_wait=[w], on_update=[])
                        out.append(nop)
                    si.on_wait = waits[-max_waits:]
                    changed = True
                out.append(inst)
            if changed:
                il[:] = out
# ---------------------------------------------------------------------------


def build_program() -> bass.Bass:
    nc = bass.Bass("TRN2", target_bir_lowering=False, debug=False)

    k_d = nc.dram_tensor("k", [128, 2, N], FP8, kind="ExternalInput").ap()
    q_d = nc.dram_tensor("q", [128, 2, MQ], FP8, kind="ExternalInput").ap()
    v_d = nc.dram_tensor("v", [128, JT // 2, 2, 256], FP8,
                         kind="ExternalInput").ap()
    h2_d = nc.dram_tensor("h2", [MQ // CHUNK, 128, 2, CHUNK], BF16,
                          kind="ExternalOutput").ap()
    den_d = nc.dram_tensor("den", [MQ // CHUNK, 1, CHUNK], F32,
                           kind="ExternalOutput").ap()

    with tile.TileContext(nc) as tc:
        with (
            tc.tile_pool(name="consts", bufs=1) as consts,
            tc.tile_pool(name="kqv", bufs=1) as kqv,
            tc.tile_pool(name="esb", bufs=14) as epool,
            tc.tile_pool(name="osb", bufs=3) as opool,
            tc.tile_pool(name="dsb", bufs=2) as dpool,
            tc.tile_pool(name="pp", bufs=3, space="PSUM") as pp,
            tc.tile_pool(name="ph2p", bufs=4, space="PSUM") as ph2p,
        ):
            ones_sb = consts.tile([128, 2, 16], FP8)
            nb_sb = consts.tile([128, 1], F32)
            nc.vector.memset(ones_sb, 1.0)
            nc.vector.memset(nb_sb, EXP_BIAS)

            kt = kqv.tile([128, 2, N], FP8)
            qt = kqv.tile([128, 2, MQ], FP8)
            vt = kqv.tile([128, JT // 2, 2, 256], FP8)
            # loads split + interleaved in chunk-0 consumption order (the
            # DMA device serializes transfers, so delivery order must track
            # the QK/AV slot order to avoid starving the first two chunks);
            # k's tail is consumed (QK slot 12) BEFORE v's tail (AV slot
            # 12+LAG) and q's tail (chunk 2). Issue round-robins over four
            # otherwise-idle engine DGE queues: one queue's ~650ns/issue
            # would outpace the small early transfers and delay delivery.
            parts = [
                (qt[:, :, 0:512], q_d[:, :, 0:512]),
                (kt[:, :, 0:1024], k_d[:, :, 0:1024]),
            ]
            for piece in range(3):
                jsl = slice(piece * (JT // 8), (piece + 1) * (JT // 8))
                ksl = slice(1024 + piece * 1792, 1024 + (piece + 1) * 1792)
                parts.append((vt[:, jsl, :, :], v_d[:, jsl, :, :]))
                parts.append((kt[:, :, ksl], k_d[:, :, ksl]))
            parts += [
                (kt[:, :, 6400:N], k_d[:, :, 6400:N]),
                (vt[:, 24:32, :, :], v_d[:, 24:32, :, :]),
                (qt[:, :, 512:MQ], q_d[:, :, 512:MQ]),
            ]
            # the first three pieces issue concurrently from idle ACT and
            # GPSIMD queues (one queue's issue rate lags the small early
            # transfers); the rest stay on SP so the serialized DMA device
            # grants them exactly in consumption order (spreading ALL
            # pieces over queues races and breaks the ordering).
            nc.scalar.dma_start(out=parts[0][0], in_=parts[0][1])
            nc.gpsimd.dma_start(out=parts[1][0], in_=parts[1][1])
            nc.scalar.dma_start(out=parts[2][0], in_=parts[2][1])
            for dst, srcap in parts[3:]:
                nc.sync.dma_start(out=dst, in_=srcap)

            DR = mybir.MatmulPerfMode.DoubleRow

            def av_den(mc, g, et, hc, den):
                # both den matmuls issue before the four AV matmuls: each
                # group's internal accumulation order is preserved, and the
                # earlier den issue measures ~200ns faster end-to-end
                for p in range(2):
                    nc.tensor.matmul(den, lhsT=ones_sb[:, :, 0:1],
                                     rhs=et[:, 2 * p:2 * p + 2, :],
                                     perf_mode=DR,
                                     start=g == 0 and p == 0,
                                     stop=g == NSLOT - 1 and p == 1)
                for p in range(2):
                    first = g == 0 and p == 0
                    last = g == NSLOT - 1 and p == 1
                    ep = et[:, 2 * p:2 * p + 2, :]
                    for ci in range(2):
                        nc.tensor.matmul(
                            hc[:, ci, :],
                            lhsT=vt[:, 2 * g + p, :,
                                    ci * 128:ci * 128 + 128],
                            rhs=ep, perf_mode=DR,
                            start=first, stop=last)

            def drain(mc, hc, den):
                # chunk mc fully accumulated: drain h2 (bf16) + den (f32)
                # and ship; the epilogue runs on the host
                h2sb = opool.tile([128, 2, CHUNK], BF16, tag="h2sb",
                                  name=f"h2sb_{mc}")
                nc.vector.tensor_copy(out=h2sb, in_=hc)
                den_sb = dpool.tile([1, CHUNK], F32, tag="densb",
                                    name=f"densb_{mc}")
                nc.scalar.activation(out=den_sb, in_=den, func=AF.Copy)
                nc.sync.dma_start(out=h2_d[mc], in_=h2sb)
                nc.sync.dma_start(out=den_d[mc], in_=den_sb)

            # one rolling pipeline over all (chunk, slot) pairs: the pend
            # queue crosses chunk boundaries so the PE always has QK work
            # while tail AVs wait on their exps. Drain emission is ALSO
            # deferred DDELAY slots past a chunk's last AV so the in-order
            # ACT/DVE engines reach the drain after its dependency cleared
            # (emitting it immediately would head-of-line-block their exps).
            pend = []
            drq = []
            hc = den = None
            for s in range(NSLOT * (MQ // CHUNK)):
                mc, g = divmod(s, NSLOT)
                if g == 0:
                    hc = ph2p.tile([128, 2, CHUNK], F32, tag="hcm", bufs=1,
                                   name=f"hc_{mc}")
                    den = ph2p.tile([1, CHUNK], F32, tag="den", bufs=1,
                                    name=f"den_{mc}")
                msl = slice(mc * CHUNK, mc * CHUNK + CHUNK)
                et = epool.tile([128, SLOT, CHUNK], FP8)
                ps4 = pp.tile([128, SLOT, CHUNK], F32, tag="ps")
                for r in range(SLOT):
                    j = g * SLOT + r
                    jsl = slice(j * 128, j * 128 + 128)
                    nc.tensor.matmul(ps4[:, r, :], lhsT=kt[:, :, jsl],
                                     rhs=qt[:, :, msl], perf_mode=DR,
                                     start=True, stop=True)
                if len(pend) >= LAG:
                    item = pend.pop(0)
                    av_den(*item)
                    if item[1] == NSLOT - 1:
                        drq.append((s, item[0], item[3], item[4]))
                if drq and s - drq[0][0] >= DDELAY:
                    _, dmc, dhc, dden = drq.pop(0)
                    drain(dmc, dhc, dden)
                if (s + S17_PHASE) % 17 in DVE_S17:
                    nc.vector.tensor_scalar(
                        out=et.bitcast(U8), in0=ps4,
                        scalar1=SCH_A, scalar2=SCH_B,
                        op0=ALU.mult, op1=ALU.add)
                else:
                    nc.scalar.activation(out=et, in_=ps4, func=AF.Exp,
                                         scale=SCALE, bias=nb_sb)
                pend.append((mc, g, et, hc, den))
            for item in pend:
                av_den(*item)
                if item[1] == NSLOT - 1:
                    drq.append((0, item[0], item[3], item[4]))
            for _, dmc, dhc, dden in drq:
                drain(dmc, dhc, dden)
    _split_excess_waits(nc)
    return nc


_NC_CACHE = None


def _get_program():
    global _NC_CACHE
    if _NC_CACHE is None:
        _NC_CACHE = build_program()
    return _NC_CACHE


def _prep_batch(inputs, b, x):
    """Fold GroupNorm (stats computed here on the host) into the q/k/v
    weights and biases for batch element b (h = s1*x + s2 per channel, so
    W @ h = (W*diag(s1)) @ x + W @ s2), then form the fp8 q/k/v operands in
    the device layouts. Returns (qkv maps per slice, wp, bp_eff); wp/bp_eff
    feed the host epilogue."""
    f32 = np.float32
    wq = np.asarray(inputs["wq"], f32)
    wk = np.asarray(inputs["wk"], f32)
    wv = np.asarray(inputs["wv"], f32)
    wp = np.asarray(inputs["wp"], f32)
    bv = np.asarray(inputs["bv"], f32)
    bp = np.asarray(inputs["bp"], f32)
    gw = np.asarray(inputs["gn_weight"], f32)
    gb = np.asarray(inputs["gn_bias"], f32)

    g = x[b].reshape(32, 8 * N)
    mean = g.mean(axis=1)
    var = g.var(axis=1)
    rstd = 1.0 / np.sqrt(var + EPS)
    s1 = np.repeat(rstd, 8) * gw                       # [C]
    s2 = gb - np.repeat(mean * rstd, 8) * gw           # [C]

    wq_f = wq * s1[None, :]
    wk_f = wk * s1[None, :]
    wv_f = wv * s1[None, :]
    bq_f = np.asarray(inputs["bq"], f32) + wq @ s2
    bk_f = np.asarray(inputs["bk"], f32) + wk @ s2
    # v's constant part rides through softmax (rows sum to 1) into the
    # host-side projection bias: bp_eff = bp + wp @ (bv + wv @ s2)
    bp_f = bp + wp @ (bv + wv @ s2)

    xb = x[b]
    k_all = wk_f @ xb + bk_f[:, None]                  # [C, N]
    q_all = wq_f @ xb + bq_f[:, None]                  # [C, N]
    v_all = (wv_f @ xb).T                              # [N, C]

    k_dev = np.ascontiguousarray(
        k_all.reshape(2, 128, N).transpose(1, 0, 2)).astype(FP8_NP)
    v_dev = np.ascontiguousarray(
        v_all.reshape(JT // 2, 2, 128, 256).transpose(2, 0, 1, 3)
    ).astype(FP8_NP)
    q8 = q_all.astype(FP8_NP)
    maps = []
    for s in range(NSLICE):
        q_dev = np.ascontiguousarray(
            q8[:, MQ * s:MQ * (s + 1)].reshape(2, 128, MQ).transpose(1, 0, 2))
        maps.append({"k": k_dev, "q": q_dev, "v": v_dev})
    return maps, wp, bp_f


def kernel(**inputs) -> np.ndarray:
    x = np.asarray(inputs["x"], np.float32)  # [B, C, N]

    in_maps = []
    wps, bps = [], []
    for b in range(B):
        maps, wp, bp_f = _prep_batch(inputs, b, x)
        wps.append(wp)
        bps.append(bp_f)
        in_maps.extend(maps)

    nc = _get_program()
    res = run_bass_kernel_spmd(nc, in_maps, core_ids=list(range(NCORES)))

    out = np.empty((B, C, N), np.float32)
    for core in range(NCORES):
        b, s = divmod(core, NSLICE)
        h2 = np.asarray(res.results[core]["h2"],
                        np.float32)             # [mc, 128, 2, CHUNK]
        h2 = h2.transpose(2, 1, 0, 3).reshape(C, MQ)  # c = 128*ci + p
        den = np.asarray(res.results[core]["den"],
                         np.float32).reshape(1, MQ)
        o = wps[b] @ (h2 / den)                               # [C, MQ]
        sl = slice(MQ * s, MQ * (s + 1))
        out[b][:, sl] = x[b][:, sl] + o + bps[b][:, None]
    return out


# revision 75
# speedup vs baseline: 1.0032x; 1.0005x over previous
"""AttnBlock (GroupNorm -> single-head attention -> proj -> residual) on 8
Trainium2 NeuronCores.

Sharding: core = (b, s); b = core // 4 selects the batch element, s = core % 4
selects a 2048-wide query slice of N=8192 (sequence-parallel queries, keys
replicated, per the problem's sharding hint). One SPMD program, static
addressing, no collectives: per-core inputs differ only in the q slice.

Work split host/device: the device runs the O(N^2) attention -- all QK^T
score matmuls, the softmax exp, the E@v context accumulation and the
denominator row -- which is >97% of the module's FLOPs and the entire
bottleneck. The host (following the baseline's GroupNorm-folding precedent)
prepares the fp8 q/k/v operands with GroupNorm folded into the 1x1-conv
weights, and applies the O(C^2 N) epilogue (normalize by den, wp
projection, bias, residual) in f32 to the device's h2/den output.

On device the softmax exp is the elementwise bottleneck (16.8M exps/core;
only ACT has table exp, and GPSIMD cannot read PSUM), so it is split
across the two PSUM-capable elementwise engines:
  ACT: table exp -> fp8e4m3             (scale=C^-1/2, bias=-2.5)
  DVE: Schraudolph bit-trick -- round(A*s + B) saturated to uint8 IS the
       e4m3 bit pattern of ~exp(s*SCALE - 2.5). The f32->uint8 saturation
       clamps the underflow tail to +0.0; RNE rounding and saturation
       verified on HW.
Both paths share the -2.5 log-bias so their scales match (den mixes tiles
from both); the bias cancels in the normalization. The bit trick adds
~+-4% noise to e, invisible next to e4m3's own mantissa quantization.

Pipeline: scores live in a ring of 3 PSUM slots ([128, 4 key-tiles, 256
queries] each); the slot-recycle chain (exp end -> QK refill -> next exp)
amortizes over the ring so the ACT/DVE exp throughput, the ring latency
and the PE matmul stream (~660ns/slot) are all balanced. AV consumption
runs LAG slots behind QK production so an in-flight exp never stalls the
in-order PE, and the pend queue rolls across chunk boundaries so chunk
tails cost nothing.
"""

import ml_dtypes
import numpy as np

import bass_rust
import concourse.bass as bass
import concourse.tile as tile
from concourse import mybir
from concourse.bass_utils import run_bass_kernel_spmd

B, C, N = 2, 256, 8192
NCORES = 8
NSLICE = 4          # query slices per batch element
MQ = N // NSLICE    # 2048 queries per core
CHUNK = 256         # queries processed per attention pass
JT = N // 128       # 64 key tiles
SLOT = 4            # key tiles per score slot: [128, 4, 256] = 2 PSUM banks
NSLOT = JT // SLOT  # 16 slots per chunk
EPS = 1e-5
SCALE = C ** -0.5   # 0.0625
EXP_BIAS = -2.5     # shared log-domain bias; cancels in normalization

# Schraudolph constants: uint8 pattern v = round(A*s + B) read as e4m3 is
# ~exp(s*SCALE + EXP_BIAS).  A = 8*SCALE/ln2;  B = 56 + 8*EXP_BIAS/ln2 - 0.344
# (the -0.344 centers the piecewise-linear 2^frac error at +-3%).
SCH_A = 8.0 * SCALE / np.log(2.0)
SCH_B = 56.0 + 8.0 * EXP_BIAS / np.log(2.0) - 0.344

# per-slot engine for the exp: ACT or DVE (GPSIMD cannot access PSUM),
# balanced by their per-slot costs (ACT ~1038ns vs DVE ~1192ns engine-busy
# per 1024-elem slot, DVE also owns the h2 drain). Keyed on the GLOBAL slot
# index mod 17 so the A/D alternation's unavoidable ACT-ACT seam drifts
# across chunks instead of compounding at chunk boundaries (a mod-16
# pattern serialized the slot ring there); the +1 phase was tuned by sweep.
DVE_S17 = frozenset({1, 3, 5, 7, 9, 11, 13, 15})   # 8/17 -> 60 of 128 slots
S17_PHASE = 1
LAG = 3     # AV of slot g issues after QK of slot g+LAG, hiding exp latency
DDELAY = 1  # chunk drains are emitted this many slots after its last AV
# With a PSUM ring of 3 score slots (6 banks) + hc packed in 1 bank + den
# 1 bank, the slot-recycle chain (exp end -> QK -> exp) amortizes over 3
# slots and the PE (~660ns/slot of matmuls) becomes the critical engine.

F32 = mybir.dt.float32
BF16 = mybir.dt.bfloat16
FP8 = mybir.dt.float8e4
U8 = mybir.dt.uint8
BF16_NP = ml_dtypes.bfloat16
FP8_NP = ml_dtypes.float8_e4m3
AF = mybir.ActivationFunctionType
ALU = mybir.AluOpType


# ---------------------------------------------------------------------------
# Workaround: this container's walrus build rejects any instruction carrying
# more than one semaphore wait ("Too many sync wait commands"). Two pieces:
# (1) the Tile exit drain gets its waits split across per-proc sync nops;
# (2) a post-pass hoists excess waits from scheduled instructions onto
#     same-engine NoOps inserted immediately before them (same engine +
#     program order => identical blocking semantics).
def _drain_and_barrier_split(self, tick_clock, wait_clock):
    gc = tick_clock.global_clock
    vals = list(gc)
    n = len(vals)
    for i, v in enumerate(vals):
        if v == 0:
            continue
        vec = [0] * n
        vec[i] = v
        nop = self.nc.sync.nop(nofuse=True, hint=f"drain_split_{i}")
        wait_clock.add_sem_waits(
            nop.ins, bass_rust.ScopedClock({None: bass_rust.VectorClock(vec)})
        )
    self.nc.sync.drain()
    self.nc.all_engine_barrier()
    assert self.sems is not None
    popped = self.nc._tile_sem_poison_stack.pop()
    assert popped is self._sem_poison
    self.nc.clear_and_free_semaphores(list(self.sems.allocated().values()))
    self.nc.all_engine_barrier()


tile.TileContext._drain_and_barrier = _drain_and_barrier_split


def _split_excess_waits(nc, max_waits=1):
    for f in nc.m.functions:
        for blk in f.blocks:
            il = blk.instructions
            out = []
            changed = False
            for inst in il:
                si = getattr(inst, "sync_info", None)
                waits = list(si.on_wait) if si is not None and si.on_wait else []
                if len(waits) > max_waits:
                    for k, w in enumerate(waits[:-max_waits]):
                        nop = bass_rust.InstNoOp(
                            name=f"{inst.name}-wsplit{k}", ins=[], outs=[])
                        nop.engine = inst.engine
                        nop.sync_info = bass_rust.SyncInfo(
                            on_wait=[w], on_update=[])
                        out.append(nop)
                    si.on_wait = waits[-max_waits:]
                    changed = True
                out.append(inst)
            if changed:
                il[:] = out
# ---------------------------------------------------------------------------


def build_program() -> bass.Bass:
    nc = bass.Bass("TRN2", target_bir_lowering=False, debug=False)

    k_d = nc.dram_tensor("k", [128, 2, N], FP8, kind="ExternalInput").ap()
    q_d = nc.dram_tensor("q", [128, 2, MQ], FP8, kind="ExternalInput").ap()
    v_d = nc.dram_tensor("v", [128, JT // 2, 2, 256], FP8,
                         kind="ExternalInput").ap()
    h2_d = nc.dram_tensor("h2", [MQ // CHUNK, 128, 2, CHUNK], BF16,
                          kind="ExternalOutput").ap()
    den_d = nc.dram_tensor("den", [MQ // CHUNK, 1, CHUNK], F32,
                           kind="ExternalOutput").ap()

    with tile.TileContext(nc) as tc:
        with (
            tc.tile_pool(name="consts", bufs=1) as consts,
            tc.tile_pool(name="kqv", bufs=1) as kqv,
            tc.tile_pool(name="esb", bufs=14) as epool,
            tc.tile_pool(name="osb", bufs=3) as opool,
            tc.tile_pool(name="dsb", bufs=2) as dpool,
            tc.tile_pool(name="pp", bufs=3, space="PSUM") as pp,
            tc.tile_pool(name="ph2p", bufs=4, space="PSUM") as ph2p,
        ):
            ones_sb = consts.tile([128, 2, 16], FP8)
            nb_sb = consts.tile([128, 1], F32)
            nc.gpsimd.memset(ones_sb, 1.0)
            nc.gpsimd.memset(nb_sb, EXP_BIAS)

            kt = kqv.tile([128, 2, N], FP8)
            qt = kqv.tile([128, 2, MQ], FP8)
            vt = kqv.tile([128, JT // 2, 2, 256], FP8)
            # loads split + interleaved in chunk-0 consumption order (the
            # DMA device serializes transfers, so delivery order must track
            # the QK/AV slot order to avoid starving the first two chunks);
            # k's tail is consumed (QK slot 12) BEFORE v's tail (AV slot
            # 12+LAG) and q's tail (chunk 2). Issue round-robins over four
            # otherwise-idle engine DGE queues: one queue's ~650ns/issue
            # would outpace the small early transfers and delay delivery.
            parts = [
                (qt[:, :, 0:512], q_d[:, :, 0:512]),
                (kt[:, :, 0:1024], k_d[:, :, 0:1024]),
            ]
            for piece in range(3):
                jsl = slice(piece * (JT // 8), (piece + 1) * (JT // 8))
                ksl = slice(1024 + piece * 1792, 1024 + (piece + 1) * 1792)
                parts.append((vt[:, jsl, :, :], v_d[:, jsl, :, :]))
                parts.append((kt[:, :, ksl], k_d[:, :, ksl]))
            parts += [
                (kt[:, :, 6400:N], k_d[:, :, 6400:N]),
                (vt[:, 24:32, :, :], v_d[:, 24:32, :, :]),
                (qt[:, :, 512:MQ], q_d[:, :, 512:MQ]),
            ]
            # the first three pieces issue concurrently from idle ACT and
            # GPSIMD queues (one queue's issue rate lags the small early
            # transfers); the rest stay on SP so the serialized DMA device
            # grants them exactly in consumption order (spreading ALL
            # pieces over queues races and breaks the ordering).
            nc.scalar.dma_start(out=parts[0][0], in_=parts[0][1])
            nc.gpsimd.dma_start(out=parts[1][0], in_=parts[1][1])
            nc.scalar.dma_start(out=parts[2][0], in_=parts[2][1])
            for dst, srcap in parts[3:]:
                nc.sync.dma_start(out=dst, in_=srcap)

            DR = mybir.MatmulPerfMode.DoubleRow

            def av_den(mc, g, et, hc, den):
                # both den matmuls issue before the four AV matmuls: each
                # group's internal accumulation order is preserved, and the
                # earlier den issue measures ~200ns faster end-to-end
                for p in range(2):
                    nc.tensor.matmul(den, lhsT=ones_sb[:, :, 0:1],
                                     rhs=et[:, 2 * p:2 * p + 2, :],
                                     perf_mode=DR,
                                     start=g == 0 and p == 0,
                                     stop=g == NSLOT - 1 and p == 1)
                for p in range(2):
                    first = g == 0 and p == 0
                    last = g == NSLOT - 1 and p == 1
                    ep = et[:, 2 * p:2 * p + 2, :]
                    for ci in range(2):
                        nc.tensor.matmul(
                            hc[:, ci, :],
                            lhsT=vt[:, 2 * g + p, :,
                                    ci * 128:ci * 128 + 128],
                            rhs=ep, perf_mode=DR,
                            start=first, stop=last)

            def drain(mc, hc, den):
                # chunk mc fully accumulated: drain h2 (bf16) + den (f32)
                # and ship; the epilogue runs on the host
                h2sb = opool.tile([128, 2, CHUNK], BF16, tag="h2sb",
                                  name=f"h2sb_{mc}")
                nc.vector.tensor_copy(out=h2sb, in_=hc)
                den_sb = dpool.tile([1, CHUNK], F32, tag="densb",
                                    name=f"densb_{mc}")
                nc.scalar.activation(out=den_sb, in_=den, func=AF.Copy)
                nc.sync.dma_start(out=h2_d[mc], in_=h2sb)
                nc.sync.dma_start(out=den_d[mc], in_=den_sb)

            # one rolling pipeline over all (chunk, slot) pairs: the pend
            # queue crosses chunk boundaries so the PE always has QK work
            # while tail AVs wait on their exps. Drain emission is ALSO
            # deferred DDELAY slots past a chunk's last AV so the in-order
            # ACT/DVE engines reach the drain after its dependency cleared
            # (emitting it immediately would head-of-line-block their exps).
            pend = []
            drq = []
            hc = den = None
            for s in range(NSLOT * (MQ // CHUNK)):
                mc, g = divmod(s, NSLOT)
                if g == 0:
                    hc = ph2p.tile([128, 2, CHUNK], F32, tag="hcm", bufs=1,
                                   name=f"hc_{mc}")
                    den = ph2p.tile([1, CHUNK], F32, tag="den", bufs=1,
                                    name=f"den_{mc}")
                msl = slice(mc * CHUNK, mc * CHUNK + CHUNK)
                et = epool.tile([128, SLOT, CHUNK], FP8)
                ps4 = pp.tile([128, SLOT, CHUNK], F32, tag="ps")
                for r in range(SLOT):
                    j = g * SLOT + r
                    jsl = slice(j * 128, j * 128 + 128)
                    nc.tensor.matmul(ps4[:, r, :], lhsT=kt[:, :, jsl],
                                     rhs=qt[:, :, msl], perf_mode=DR,
                                     start=True, stop=True)
                if len(pend) >= LAG:
                    item = pend.pop(0)
                    av_den(*item)
                    if item[1] == NSLOT - 1:
                        drq.append((s, item[0], item[3], item[4]))
                if drq and s - drq[0][0] >= DDELAY:
                    _, dmc, dhc, dden = drq.pop(0)
                    drain(dmc, dhc, dden)
                if (s + S17_PHASE) % 17 in DVE_S17:
                    nc.vector.tensor_scalar(
                        out=et.bitcast(U8), in0=ps4,
                        scalar1=SCH_A, scalar2=SCH_B,
                        op0=ALU.mult, op1=ALU.add)
                else:
                    nc.scalar.activation(out=et, in_=ps4, func=AF.Exp,
                                         scale=SCALE, bias=nb_sb)
                pend.append((mc, g, et, hc, den))
            for item in pend:
                av_den(*item)
                if item[1] == NSLOT - 1:
                    drq.append((0, item[0], item[3], item[4]))
            for _, dmc, dhc, dden in drq:
                drain(dmc, dhc, dden)
    _split_excess_waits(nc)
    return nc


_NC_CACHE = None


def _get_program():
    global _NC_CACHE
    if _NC_CACHE is None:
        _NC_CACHE = build_program()
    return _NC_CACHE


def _prep_batch(inputs, b, x):
    """Fold GroupNorm (stats computed here on the host) into the q/k/v
    weights and biases for batch element b (h = s1*x + s2 per channel, so
    W @ h = (W*diag(s1)) @ x + W @ s2), then form the fp8 q/k/v operands in
    the device layouts. Returns (qkv maps per slice, wp, bp_eff); wp/bp_eff
    feed the host epilogue."""
    f32 = np.float32
    wq = np.asarray(inputs["wq"], f32)
    wk = np.asarray(inputs["wk"], f32)
    wv = np.asarray(inputs["wv"], f32)
    wp = np.asarray(inputs["wp"], f32)
    bv = np.asarray(inputs["bv"], f32)
    bp = np.asarray(inputs["bp"], f32)
    gw = np.asarray(inputs["gn_weight"], f32)
    gb = np.asarray(inputs["gn_bias"], f32)

    g = x[b].reshape(32, 8 * N)
    mean = g.mean(axis=1)
    var = g.var(axis=1)
    rstd = 1.0 / np.sqrt(var + EPS)
    s1 = np.repeat(rstd, 8) * gw                       # [C]
    s2 = gb - np.repeat(mean * rstd, 8) * gw           # [C]

    wq_f = wq * s1[None, :]
    wk_f = wk * s1[None, :]
    wv_f = wv * s1[None, :]
    bq_f = np.asarray(inputs["bq"], f32) + wq @ s2
    bk_f = np.asarray(inputs["bk"], f32) + wk @ s2
    # v's constant part rides through softmax (rows sum to 1) into the
    # host-side projection bias: bp_eff = bp + wp @ (bv + wv @ s2)
    bp_f = bp + wp @ (bv + wv @ s2)

    xb = x[b]
    k_all = wk_f @ xb + bk_f[:, None]                  # [C, N]
    q_all = wq_f @ xb + bq_f[:, None]                  # [C, N]
    v_all = (wv_f @ xb).T                              # [N, C]

    k_dev = np.ascontiguousarray(
        k_all.reshape(2, 128, N).transpose(1, 0, 2)).astype(FP8_NP)
    v_dev = np.ascontiguousarray(
        v_all.reshape(JT // 2, 2, 128, 256).transpose(2, 0, 1, 3)
    ).astype(FP8_NP)
    q8 = q_all.astype(FP8_NP)
    maps = []
    for s in range(NSLICE):
        q_dev = np.ascontiguousarray(
            q8[:, MQ * s:MQ * (s + 1)].reshape(2, 128, MQ).transpose(1, 0, 2))
        maps.append({"k": k_dev, "q": q_dev, "v": v_dev})
    return maps, wp, bp_f


def kernel(**inputs) -> np.ndarray:
    x = np.asarray(inputs["x"], np.float32)  # [B, C, N]

    in_maps = []
    wps, bps = [], []
    for b in range(B):
        maps, wp, bp_f = _prep_batch(inputs, b, x)
        wps.append(wp)
        bps.append(bp_f)
        in_maps.extend(maps)

    nc = _get_program()
    res = run_bass_kernel_spmd(nc, in_maps, core_ids=list(range(NCORES)))

    out = np.empty((B, C, N), np.float32)
    for core in range(NCORES):
        b, s = divmod(core, NSLICE)
        h2 = np.asarray(res.results[core]["h2"],
                        np.float32)             # [mc, 128, 2, CHUNK]
        h2 = h2.transpose(2, 1, 0, 3).reshape(C, MQ)  # c = 128*ci + p
        den = np.asarray(res.results[core]["den"],
                         np.float32).reshape(1, MQ)
        o = wps[b] @ (h2 / den)                               # [C, MQ]
        sl = slice(MQ * s, MQ * (s + 1))
        out[b][:, sl] = x[b][:, sl] + o + bps[b][:, None]
    return out
